# revision 4
# baseline (speedup 1.0000x reference)
"""Trainium2 Bass kernel for BiochemicalDynamics.

Reference computation (f32):
    Ax    = A @ x                                   # [N, DIM]
    s     = R * rowsum(x * Ax)                      # [N, 1]
    out   = F - B*x - s                             # [N, DIM]

Design (v4): compute Y = (A_c @ x)^T on the TensorEngine directly.
Each core holds A_c = A[rows_c, :] shipped as fp8(e4m3) A_c^T tiles
("bt").  For each 128-row j-block a matmul with stationary xs[jblock]
(fp8 x) and moving bt[jblock] accumulates Y[d, i] += sum_j x[j,d]*A[i,j]
in PSUM.  This keeps the per-element A work on the PE (fastest engine)
instead of the DVE (the old bottleneck) and halves HBM traffic vs fp16.

Column-tiling: even j-blocks run at tile_position (0,0) -> PSUM
partitions 0..63, odd j-blocks at (0,64) -> partitions 64..127; the two
streams execute concurrently in the PE array.  The partition split is
free: the final dot already sums over partitions (via a -1s-stationary
matmul).

The kernel runs as four independent pipelines over output-column
quarters (i in [256q, 256q+256)): each quarter streams its ~2.1MB of
bt, accumulates Y_q, then D_q = (R x^T (.) Y_q) on the DVE, a reduction
matmul into OutP (seeded early with -B*x^T via a -B*I stationary
matmul), a ScalarE Copy (+F bias) and the output DMA.  Quarters 0-2's
epilogues overlap the remaining stream; only quarter 3's epilogue is
serial tail (and its last DMA chunks ramp down to 64KB).

The bt stream is split across BOTH HWDGE rings (Sync + Scalar) —
measured single-queue rate is ~300 GB/s while two concurrent queues
reach ~365 GB/s aggregate.

A burst of warm-up matmuls on a memset scratch tile runs during the
otherwise-dead framework preamble (~5us) so the PE's HAM clock gate is
already at 8/8 (2.4 GHz) when the real matmul stream begins.

Sharding: row-shard A across the 8 cores; every core gets the full x
(host-side replication).  No cross-core communication.
"""

import sys

import numpy as np

for _p in ("/opt/trn_rl_repo", "/root/.axon_site/_ro/trn_rl_repo"):
    if _p not in sys.path:
        sys.path.append(_p)

N = 8192
DIM = 64
NCORES = 8
ROWS = N // NCORES       # 1024 rows of A (and output) per core

F_CONST = 1.0
B_CONST = 0.1
R_CONST = 0.01

P = 128                  # SBUF partitions
NBLK = N // P            # 64 j-blocks
HALF = 256               # output-column slice width
NH = ROWS // HALF        # 4 slices
HBYTES = NBLK * HALF     # fp8 bytes per slice per partition

# bt DMA chunks per quarter, in j-blocks (block-tile = 32KB fp8).
# 1MB steady chunks; the last quarter ramps down so the final matmuls
# wait on tiny transfers.
BT_CHUNKS = [[32, 32], [32, 32], [32, 32], [32, 16, 8, 4, 2, 2]]
assert all(sum(c) == NBLK for c in BT_CHUNKS)

N_WARM = 16              # warm-up matmuls (~3.4us cold) to trip HAM

_CACHE = {}


def _build_nc():
    import concourse.mybir as mybir
    import concourse.tile as tile
    from concourse import bacc

    f32 = mybir.dt.float32
    bf16 = mybir.dt.bfloat16
    f8 = mybir.dt.float8e4

    nc = bacc.Bacc(
        trn_type="TRN2", target_bir_lowering=False, debug=False, num_devices=NCORES
    )

    # A^T blocks, fp8: bt[p, q*HBYTES + b*HALF + i'] = A[rows_c[HALF*q+i'], 128b+p]
    bt = nc.dram_tensor("bt", [P, NH * HBYTES], f8, kind="ExternalInput")
    # x stationaries, fp8: xs[p, 64*b + d] = x[128*b + p, d]
    xs = nc.dram_tensor("xs", [P, NBLK * DIM], f8, kind="ExternalInput")
    # [R*xloc^T; R*xloc^T] duplicated, bf16
    xtr2 = nc.dram_tensor("xtr2", [P, ROWS], bf16, kind="ExternalInput")
    # xloc^T bf16 (moving operand of the -B*x seed matmul)
    xtb = nc.dram_tensor("xtb", [DIM, ROWS], bf16, kind="ExternalInput")
    # -1s [128, 64] and -B*I64, bf16 stationaries
    wneg = nc.dram_tensor("wneg", [P, DIM], bf16, kind="ExternalInput")
    wbi = nc.dram_tensor("wbi", [DIM, DIM], bf16, kind="ExternalInput")
    # out^T bf16: out[d, i] = F - B*xloc[i, d] - s_i
    out = nc.dram_tensor("out", [DIM, ROWS], bf16, kind="ExternalOutput")

    mult = mybir.AluOpType.mult

    with tile.TileContext(nc) as tc:
        with (
            tc.tile_pool(name="big", bufs=1) as big,
            tc.tile_pool(name="small", bufs=1) as small,
            tc.tile_pool(name="psum", bufs=1, space="PSUM") as psum_pool,
        ):
            # --- PE warm-up on a memset scratch tile (no input deps) ---
            scr = small.tile([P, 256], f32)
            nc.vector.memset(scr[:], 1.0)
            warm_ps = psum_pool.tile([DIM, 256], f32, tag="warm")
            for _ in range(N_WARM):
                nc.tensor.matmul(
                    warm_ps[:], scr[:, :DIM], scr[:], start=True, stop=True
                )

            # --- x-side loads, first on the Scalar (ACT) HWDGE ring ---
            wbi_sb = small.tile([DIM, DIM], bf16)
            nc.scalar.dma_start(out=wbi_sb[:], in_=wbi[:])
            xtb_sb = small.tile([DIM, ROWS], bf16)
            nc.scalar.dma_start(out=xtb_sb[:], in_=xtb[:])
            xs_sb = small.tile([P, NBLK * DIM], f8)
            for o, w in ((0, 32 * DIM), (32 * DIM, 32 * DIM)):
                nc.scalar.dma_start(out=xs_sb[:, o : o + w], in_=xs[:, o : o + w])
            xtr2_sb = small.tile([P, ROWS], bf16)
            nc.scalar.dma_start(out=xtr2_sb[:], in_=xtr2[:])
            wneg_sb = small.tile([P, DIM], bf16)
            nc.scalar.dma_start(out=wneg_sb[:], in_=wneg[:])

            # --- A^T stream split across both HWDGE rings ---
            # Sync takes the first 4 chunks (scalar is busy with x-side),
            # then chunks alternate scalar/sync.
            bt_sb = big.tile([P, NH * HBYTES], f8)
            chunk_list = []
            for q in range(NH):
                boff = 0
                for nb in BT_CHUNKS[q]:
                    o = q * HBYTES + boff * HALF
                    chunk_list.append((o, nb * HALF))
                    boff += nb
            for idx, (o, w) in enumerate(chunk_list):
                eng = nc.sync if (idx < 4 or idx % 2 == 1) else nc.scalar
                eng.dma_start(out=bt_sb[:, o : o + w], in_=bt[:, o : o + w])

            # Output accumulator [64, 1024] f32 (2 PSUM banks).
            outp = psum_pool.tile([DIM, ROWS], f32, tag="outp")
            # Seeds: OutP[:, q] = -B * xloc^T[:, q]   (start=True clears)
            for q in range(NH):
                nc.tensor.matmul(
                    outp[:, q * HALF : (q + 1) * HALF],
                    wbi_sb[:],
                    xtb_sb[:, q * HALF : (q + 1) * HALF],
                    start=True, stop=False,
                )

            o_sb = small.tile([DIM, ROWS], bf16)
            for q in range(NH):
                # Y_q [128, 256] f32: partitions 0..63 take even j-blocks
                # (col group 0), 64..127 odd (col group 64); the matmul
                # pairs run concurrently in the PE array.
                y = psum_pool.tile([P, HALF], f32, tag="y", bufs=2)
                for k in range(NBLK // 2):
                    for half, b in ((0, 2 * k), (1, 2 * k + 1)):
                        nc.tensor.matmul(
                            y[half * DIM : (half + 1) * DIM, :],
                            xs_sb[:, b * DIM : (b + 1) * DIM],
                            bt_sb[
                                :,
                                q * HBYTES + b * HALF : q * HBYTES + (b + 1) * HALF,
                            ],
                            start=(k == 0), stop=(k == NBLK // 2 - 1),
                            tile_position=(0, half * DIM),
                        )

                # D = (R*x^T) (.) Y  on the DVE (the only sizable DVE op)
                d_sb = small.tile([P, HALF], bf16, tag="d", bufs=2)
                nc.vector.scalar_tensor_tensor(
                    d_sb[:],
                    xtr2_sb[:, q * HALF : (q + 1) * HALF],
                    1.0,
                    y[:],
                    op0=mult, op1=mult,
                )
                # OutP[:, q] -= sum_p D[p, :]  (matmul with -1s stationary)
                nc.tensor.matmul(
                    outp[:, q * HALF : (q + 1) * HALF],
                    wneg_sb[:],
                    d_sb[:],
                    start=False, stop=True,
                )
                # out^T[:, q] = OutP[:, q] + F  (ScalarE, PSUM->SBUF, bf16)
                nc.scalar.activation(
                    o_sb[:, q * HALF : (q + 1) * HALF],
                    outp[:, q * HALF : (q + 1) * HALF],
                    mybir.ActivationFunctionType.Copy,
                    bias=F_CONST, scale=1.0,
                )
                nc.sync.dma_start(
                    out=out[:, q * HALF : (q + 1) * HALF],
                    in_=o_sb[:, q * HALF : (q + 1) * HALF],
                )

    nc.finalize()
    return nc


def _get_nc():
    if "nc" not in _CACHE:
        _CACHE["nc"] = _build_nc()
    return _CACHE["nc"]


def _make_in_maps(x, A):
    import ml_dtypes

    bf16 = ml_dtypes.bfloat16
    f8 = ml_dtypes.float8_e4m3
    x = np.ascontiguousarray(np.asarray(x, dtype=np.float32))
    A = np.ascontiguousarray(np.asarray(A, dtype=np.float32))

    x8 = x.astype(f8)
    # xs[p, 64*b + d] = x8[128*b + p, d]
    xs = np.ascontiguousarray(
        x8.reshape(NBLK, P, DIM).transpose(1, 0, 2)
    ).reshape(P, NBLK * DIM)
    wneg = np.full((P, DIM), -1.0, dtype=bf16)
    wbi = (-B_CONST * np.eye(DIM, dtype=np.float32)).astype(bf16)

    in_maps = []
    for c in range(NCORES):
        rows = slice(c * ROWS, (c + 1) * ROWS)
        a8 = A[rows].astype(f8)  # [1024, 8192]
        # bt[p, q*HBYTES + b*HALF + i'] = a8[HALF*q + i', 128b + p]
        bt = np.ascontiguousarray(
            a8.reshape(NH, HALF, NBLK, P).transpose(3, 0, 2, 1)
        ).reshape(P, NH * HBYTES)
        xloc = x[rows]                      # [1024, 64] f32
        xt = np.ascontiguousarray(xloc.T)   # [64, 1024]
        xtr2 = np.ascontiguousarray(
            np.vstack([R_CONST * xt, R_CONST * xt]).astype(bf16)
        )
        in_maps.append(
            {
                "bt": bt,
                "xs": xs,
                "xtr2": xtr2,
                "xtb": xt.astype(bf16),
                "wneg": wneg,
                "wbi": wbi,
            }
        )
    return in_maps


def run_sharded(x, A, trace=False, **kwargs):
    """Run the SPMD bass kernel; returns (full_output, BassKernelResults)."""
    from concourse.bass_utils import run_bass_kernel_spmd

    nc = _get_nc()
    res = run_bass_kernel_spmd(
        nc, _make_in_maps(x, A), core_ids=list(range(NCORES)), trace=trace, **kwargs
    )
    # out is [64, 1024] bf16 per core -> [1024, 64] f32, concatenated
    full = np.concatenate(
        [res.results[c]["out"].astype(np.float32).T for c in range(NCORES)], axis=0
    )
    return np.ascontiguousarray(full), res


def kernel(t, x, A):
    out, _ = run_sharded(x, A)
    return out
